# revision 15
# baseline (speedup 1.0000x reference)
"""Causal self-attention (B=2, S=2048, E=1024, H=16) on 8 Trainium2 cores.

Sharding: core c in 0..7 handles batch b = c//4 and the 4 heads
[4*(c%4), 4*(c%4)+4).  The host pre-transposes x[b] and pre-slices the
QKV weights column-wise / Wo row-wise per core (already in the SBUF
partition-major layout the kernel wants); each core computes its heads'
attention plus its partial output projection, and the host sums the 4
f16 partials per batch.

Device kernel (per core, everything resident in SBUF, matmul inputs in
fp16 with fp32 PSUM accumulation):
  Inputs stream in over three DMA queues (x col-blocks alternate SP/ACT
  HWDGE, weights via Pool SWDGE, first e-chunk first) so the first
  projection chain starts ~3us in and PE never re-enters the slow
  p-state: wave 0 runs 8 concurrent QT/KT chains (q-blocks 0+1,
  borrowing the idle attention PSUM banks) ec-major against the
  arriving x blocks, then V[0..3].
  S^T tiles = matmul(lhsT=KT_blk, rhs=QT_blk): k on partitions, q free;
  exp on ScalarE (1/sqrt(D) folded into the activation scale); causal
  masking = never computing strictly-below-diagonal column ranges plus
  one 128x128 triangular mask multiply per diagonal block.
  P^T @ V with V augmented by a ones column (softmax denominator falls
  out of the same accumulation); normalization = copy O^T|l to SBUF
  (frees PSUM early), reciprocal on DVE, partition-broadcast of 1/l on
  the otherwise-idle Pool engine, one DVE multiply.
  Y = O @ Wo streamed out per q-block as f16 (host upcasts and sums).
  Fill (remaining waves / prev Y chains) is deficit-paced between the S
  and PV pairs of every kb-step, with an extra dose at head-pair
  boundaries to cover the O-PSUM handoff; the last q-block's Y is split
  so its first half runs as late-attention fill and two Y(2) s-blocks
  are held back to cover the final normalize chain.
"""

import numpy as np
from contextlib import ExitStack

B, S, E, H, D = 2, 2048, 1024, 16, 64
N_CORES = 8
CPB = 4              # cores per batch
HL = H // CPB        # heads per core = 4
DL = HL * D          # local head dims = 256
P = 128              # partitions
EC = E // P          # 8 e-chunks
SB = S // P          # 16 s/k blocks
NQB = S // 512       # 4 q blocks of 512
MT = DL // P         # 2 row-tiles of QT/KT/OT (2 heads each)

_CACHE = {}
_EXHAUSTED = object()

# rough PE-ns unit costs for fill pacing (matmul ~ N * 0.4167ns)
_NS_MM512 = 213
_NS_MM256 = 107


def _emit(ctx, tc, xT, wq, wk, wv, wo, mask, y, y2, loop_n=0):
    import concourse.bass as bass  # noqa: F401
    from concourse import mybir

    nc = tc.nc
    f32 = mybir.dt.float32
    f16 = mybir.dt.float16
    Exp = mybir.ActivationFunctionType.Exp

    res = ctx.enter_context(tc.tile_pool(name="res", bufs=1))
    xt_sb = res.tile([P, EC, S], f16, tag="xt")
    wq_sb = res.tile([P, EC, DL], f16, tag="wq")
    wk_sb = res.tile([P, EC, DL], f16, tag="wk")
    wv_sb = res.tile([P, EC, DL], f16, tag="wv")
    wo_sb = res.tile([P, MT, E], f16, tag="wo")
    qt_sb = res.tile([P, MT, S], f16, tag="qt")
    kt_sb = res.tile([P, MT, S], f16, tag="kt")
    vt_sb = res.tile([P, SB, HL, D + 1], f16, tag="vt")
    ot_sb = res.tile([P, MT, S], f16, tag="ot")
    ot3_sb = res.tile([P, MT, 512], f16, tag="ot3")
    mask_sb = res.tile([P, P], f16, tag="mask")

    mm_ps = ctx.enter_context(tc.tile_pool(name="mm", bufs=2, space="PSUM"))
    s_ps = ctx.enter_context(tc.tile_pool(name="sps", bufs=2, space="PSUM"))
    o_ps = ctx.enter_context(tc.tile_pool(name="ops", bufs=2, space="PSUM"))

    e_pool = ctx.enter_context(tc.tile_pool(name="ep", bufs=4))
    y_pool = ctx.enter_context(tc.tile_pool(name="yp", bufs=8))
    n_pool = ctx.enter_context(tc.tile_pool(name="np", bufs=8))

    def _full_body():
        sp, act, gp = nc.sync, nc.scalar, nc.gpsimd
        tail_norm = {}

        # ---- input DMAs: weights on Pool SWDGE (first e-chunks first so
        # the projection chains can start), x col-blocks on SP/ACT HWDGE,
        # ec-major for q-blocks 0/1 feeding wave 0, then q-blocks 2/3. ----
        act.dma_start(out=wk_sb[:, 0:1, :], in_=wk[:, 0:1, :])
        act.dma_start(out=wk_sb[:, 1:4, :], in_=wk[:, 1:4, :])
        gp.dma_start(out=wq_sb[:, 0:1, :], in_=wq[:, 0:1, :])
        gp.dma_start(out=wq_sb[:, 1:4, :], in_=wq[:, 1:4, :])
        gp.dma_start(out=wq_sb[:, 4:8, :], in_=wq[:, 4:8, :])
        gp.dma_start(out=wk_sb[:, 4:8, :], in_=wk[:, 4:8, :])
        gp.dma_start(out=wv_sb[:], in_=wv[:])
        gp.dma_start(out=mask_sb[:], in_=mask[:])
        gp.dma_start(out=wo_sb[:], in_=wo[:])
        for ec in range(EC):
            sp.dma_start(out=xt_sb[:, ec, 0:512], in_=xT[:, ec, 0:512])
            act.dma_start(out=xt_sb[:, ec, 512:1024], in_=xT[:, ec, 512:1024])
        for nb in (2, 3):
            for ec in range(EC):
                eng = sp if ec % 2 == 0 else act
                eng.dma_start(out=xt_sb[:, ec, nb * 512:(nb + 1) * 512],
                              in_=xT[:, ec, nb * 512:(nb + 1) * 512])
        nc.vector.memset(vt_sb[:, :, :, D:D + 1], 1.0)

        def wave_units(nb, parts=("qt", "kt", "v")):
            # QT/KT [:, :, nb-window] = (w chunk)^T @ xT ; V[4nb..4nb+3].
            # Generator yielding PE-ns cost estimates between chunks so the
            # attention weave can deficit-pace its fill.
            srcs = []
            if "qt" in parts:
                srcs.append((wq_sb, qt_sb))
            if "kt" in parts:
                srcs.append((wk_sb, kt_sb))
            for mt in range(MT):
                for w_sb, t_sb in srcs:
                    ps = mm_ps.tile([P, 512], f32, tag="mm", name="wv_ps")
                    for ec in range(EC):
                        nc.tensor.matmul(
                            ps[:],
                            w_sb[:, ec, mt * P:(mt + 1) * P],
                            xt_sb[:, ec, nb * 512:(nb + 1) * 512],
                            start=(ec == 0), stop=(ec == EC - 1))
                        if ec == 3:
                            yield 4 * _NS_MM512
                    nc.vector.tensor_copy(
                        t_sb[:, mt, nb * 512:(nb + 1) * 512], ps[:])
                    yield 4 * _NS_MM512
            if "v" not in parts:
                return
            for sb in range(4 * nb, 4 * nb + 4):
                ps = mm_ps.tile([P, 512], f32, tag="mm", name="v_ps")
                for ec in range(EC):
                    nc.tensor.matmul(
                        ps[:, 0:DL],
                        xt_sb[:, ec, sb * P:(sb + 1) * P],
                        wv_sb[:, ec, :],
                        start=(ec == 0), stop=(ec == EC - 1))
                    if ec == 3:
                        yield 4 * _NS_MM256
                nc.vector.tensor_copy(
                    vt_sb[:, sb, :, 0:D],
                    ps[:, 0:DL].rearrange("p (h d) -> p h d", h=HL))
                yield 4 * _NS_MM256

        def out_proj_units(qb, sbs=None):
            # Y[sb, :] = O[sb, :] @ wo, staged to f16, one DMA per s-block.
            if sbs is None:
                sbs = range(4 * qb, 4 * qb + 4)
            for sb in sbs:
                yt = y_pool.tile([P, E], f16, tag="y", name="yt")
                for eb in range(E // 512):
                    yp = mm_ps.tile([P, 512], f32, tag="mm", name="yp")
                    for dc in range(MT):
                        nc.tensor.matmul(
                            yp[:],
                            ot_sb[:, dc, sb * P:(sb + 1) * P],
                            wo_sb[:, dc, eb * 512:(eb + 1) * 512],
                            start=(dc == 0), stop=(dc == MT - 1))
                    nc.vector.tensor_copy(
                        yt[:, eb * 512:(eb + 1) * 512], yp[:])
                    yield 2 * _NS_MM512
                sp.dma_start(out=y[sb * P:(sb + 1) * P, :], in_=yt[:])

        def y3_dc0_units():
            # head-pair-0 half of the last q-block's output projection
            # (ready after attn(3)'s first mt pass) ships to the y2 partial
            # output, which the host sums into y; only the head-pair-1
            # matmuls remain for the tail.
            for sb in range(12, 16):
                for eb in range(2):
                    yt = y_pool.tile([P, 512], f16, tag="y", name="y2t")
                    yp = mm_ps.tile([P, 512], f32, tag="mm", name="yp0")
                    nc.tensor.matmul(
                        yp[:],
                        ot3_sb[:, 0, (sb - 12) * P:(sb - 11) * P],
                        wo_sb[:, 0, eb * 512:(eb + 1) * 512],
                        start=True, stop=True)
                    nc.vector.tensor_copy(yt[:], yp[:])
                    sp.dma_start(
                        out=y2[(sb - 12) * P:(sb - 11) * P,
                               eb * 512:(eb + 1) * 512],
                        in_=yt[:])
                    yield _NS_MM512

        def y3_dc1_units():
            Copy = mybir.ActivationFunctionType.Copy
            ops3, rbcs = tail_norm["ops"], tail_norm["rbcs"]
            for sb in range(12, 16):
                cs = slice((sb - 12) * P, (sb - 11) * P)
                for half in range(2):
                    dr = half * D
                    nc.vector.tensor_mul(
                        ot3_sb[dr:dr + D, 1, cs],
                        ops3[half][0:D, cs], rbcs[half][0:D, cs])
                yt = y_pool.tile([P, E], f16, tag="y", name="yt3")
                for eb in range(2):
                    yp = mm_ps.tile([P, 512], f32, tag="mm", name="yp1")
                    nc.tensor.matmul(
                        yp[:],
                        ot3_sb[:, 1, (sb - 12) * P:(sb - 11) * P],
                        wo_sb[:, 1, eb * 512:(eb + 1) * 512],
                        start=True, stop=True)
                    dst = yt[:, eb * 512:(eb + 1) * 512]
                    if eb == 0:
                        nc.scalar.activation(out=dst, in_=yp[:], func=Copy)
                    else:
                        nc.vector.tensor_copy(dst, yp[:])
                    sp.dma_start(
                        out=y[sb * P:(sb + 1) * P, eb * 512:(eb + 1) * 512],
                        in_=dst)
                    yield _NS_MM512

        def attention_block(qb, fill_units, fill_ns, boosts=()):
            # ACT-paced kb-steps; fill (PE work with no dependence on this
            # block) is inserted between the S pair and the PV pair of every
            # step, deficit-paced so the reservoir lasts the whole block;
            # head-pair boundaries get an extra dose to cover the O-PSUM
            # handoff, and `boosts` front-loads deadline-constrained units
            # (e.g. this block's own late V tiles).
            nkb = 4 * (qb + 1)     # causal: k blocks 0 .. nkb-1
            scale = float(1.0 / np.sqrt(D))
            nsteps = MT * nkb
            boosts = dict(boosts)
            committed = MT * 500 + MT * 2 * 300 + sum(boosts.values())
            per_step = max(0.0, fill_ns - committed) / max(1, nsteps - 4)
            state = {"due": 0.0, "done": False}

            def run_fill(budget):
                state["due"] += budget
                while state["due"] > 0 and not state["done"]:
                    got = next(fill_units, _EXHAUSTED)
                    if got is _EXHAUSTED:
                        state["done"] = True
                        break
                    state["due"] -= got

            for mt in range(MT):   # head pair (2*mt, 2*mt+1)
                op0 = o_ps.tile([P, 512], f32, tag="o", name="op0")
                op1 = o_ps.tile([P, 512], f32, tag="o", name="op1")
                ops = [op0, op1]
                for kb in range(nkb):
                    t = kb - 4 * qb
                    v0 = P * t if t > 0 else 0   # masked prefix of window
                    sp_t = s_ps.tile([P, 1024], f32, tag="s", name="sp_t")
                    for half in range(2):
                        dr = half * D
                        nc.tensor.matmul(
                            sp_t[:, half * 512 + v0:(half + 1) * 512],
                            kt_sb[dr:dr + D, mt, kb * P:(kb + 1) * P],
                            qt_sb[dr:dr + D, mt,
                                  qb * 512 + v0:(qb + 1) * 512],
                            start=True, stop=True)
                    et = e_pool.tile([P, 1024], f16, tag="e", name="et")
                    nc.scalar.activation(out=et[:, v0:], in_=sp_t[:, v0:],
                                         func=Exp, scale=scale)
                    if t >= 0:  # diagonal block: mask strictly-future keys
                        for half in range(2):
                            w0 = half * 512 + v0
                            nc.vector.tensor_mul(
                                et[:, w0:w0 + P], et[:, w0:w0 + P],
                                mask_sb[:])
                    step = mt * nkb + kb
                    run_fill(per_step + (500 if kb == 0 else 0)
                             + boosts.get(step, 0))
                    for half in range(2):
                        nc.tensor.matmul(
                            ops[half][0:D + 1, v0:],
                            vt_sb[:, kb, 2 * mt + half, :],
                            et[:, half * 512 + v0:(half + 1) * 512],
                            start=(kb == 0), stop=(kb == nkb - 1))
                # normalize: copy O^T|l off PSUM (frees the o tiles fast),
                # 1/l on DVE, partition-broadcast on Pool, one DVE multiply.
                last = (qb == NQB - 1)
                if last and mt == MT - 1:
                    # tail-critical: 1/l straight from the O PSUM (held to
                    # the end -- nothing else needs the banks), partition-
                    # broadcast to SBUF on Pool; the per-s-block muls are
                    # emitted by the dc1 units so each unblocks as soon as
                    # its own columns are normalized.
                    rbcs = []
                    for half in range(2):
                        rec = n_pool.tile([1, 512], f32, tag="rec",
                                          name="rec")
                        nc.vector.reciprocal(rec[:], ops[half][D:D + 1, :])
                        rbc = n_pool.tile([D, 512], f32, tag="rbc",
                                          name="rbc")
                        nc.gpsimd.partition_broadcast(rbc[:], rec[:])
                        rbcs.append(rbc)
                    tail_norm["ops"] = ops
                    tail_norm["rbcs"] = rbcs
                    continue
                for half in range(2):
                    op = ops[half]
                    dr = half * D
                    otr = n_pool.tile([D + 1, 512], f32, tag="otr",
                                      name="otr")
                    if last:
                        nc.scalar.activation(
                            out=otr[:], in_=op[0:D + 1, :],
                            func=mybir.ActivationFunctionType.Copy)
                    else:
                        nc.vector.tensor_copy(otr[:], op[0:D + 1, :])
                    rec = n_pool.tile([1, 512], f32, tag="rec", name="rec")
                    nc.vector.reciprocal(rec[:], otr[D:D + 1, :])
                    rbc = n_pool.tile([D, 512], f32, tag="rbc", name="rbc")
                    nc.gpsimd.partition_broadcast(rbc[:], rec[:])
                    if last:
                        dst = ot3_sb[dr:dr + D, mt, :]
                    else:
                        dst = ot_sb[dr:dr + D, mt, qb * 512:(qb + 1) * 512]
                    nc.vector.tensor_mul(dst, otr[0:D, :], rbc[:])
                    run_fill(300)
            # drain leftover fill
            while next(fill_units, _EXHAUSTED) is not _EXHAUSTED:
                pass

        # ---- wave 0, ec-major: 8 concurrent QT/KT chains (q-blocks 0+1;
        # accumulators borrowed from every PSUM pool) advance one e-chunk
        # at a time as x col-blocks land, keeping PE in deep backlog so the
        # p-state ramps once, then V[0..3].  q-block-0 copies retire on DVE
        # as their chains stop; the rest split across DVE and the
        # still-idle ACT engine so vt copies are never stuck behind them.
        Copy = mybir.ActivationFunctionType.Copy
        w0ps = []
        for mt in range(MT):
            w0ps.append((0, mt, wq_sb, qt_sb,
                         mm_ps.tile([P, 512], f32, tag="mm", name="w0qa")))
        for mt in range(MT):
            w0ps.append((0, mt, wk_sb, kt_sb,
                         o_ps.tile([P, 512], f32, tag="o", name="w0ka")))
        sA = s_ps.tile([P, 1024], f32, tag="s", name="w0sA")
        sB = s_ps.tile([P, 1024], f32, tag="s", name="w0sB")
        for mt in range(MT):
            w0ps.append((1, mt, wq_sb, qt_sb, sA[:, mt * 512:(mt + 1) * 512]))
        for mt in range(MT):
            w0ps.append((1, mt, wk_sb, kt_sb, sB[:, mt * 512:(mt + 1) * 512]))
        for ec in range(EC):
            for i, (nb, mt, w_sb, t_sb, pchain) in enumerate(w0ps):
                nc.tensor.matmul(
                    pchain[:],
                    w_sb[:, ec, mt * P:(mt + 1) * P],
                    xt_sb[:, ec, nb * 512:(nb + 1) * 512],
                    start=(ec == 0), stop=(ec == EC - 1))
                if ec == EC - 1:
                    nb_, mt_, _, t_sb_, pch_ = w0ps[i]
                    dst = t_sb_[:, mt_, nb_ * 512:(nb_ + 1) * 512]
                    if i in (0, 1, 4, 5):
                        nc.vector.tensor_copy(dst, pch_[:])
                    else:
                        nc.scalar.activation(out=dst, in_=pch_[:], func=Copy)
        for sb in range(4):
            ps = mm_ps.tile([P, 512], f32, tag="mm", name="w0v")
            for ec in range(EC):
                nc.tensor.matmul(
                    ps[:, 0:DL],
                    xt_sb[:, ec, sb * P:(sb + 1) * P],
                    wv_sb[:, ec, :],
                    start=(ec == 0), stop=(ec == EC - 1))
            nc.vector.tensor_copy(
                vt_sb[:, sb, :, 0:D],
                ps[:, 0:DL].rearrange("p (h d) -> p h d", h=HL))

        # Fill plan, sized to each block's ACT-over-PE deficit (which grows
        # with qb): attention(3) gets its own late V tiles (front-loaded via
        # boosts to beat the kb=12 deadline), Y(1), the first half of Y(2),
        # and Y(3)'s dc0 partials; Y(2)'s last two s-blocks plus the dc1
        # finish run in the tail so the final normalize chain has PE work
        # under it.
        qk_ns = 4 * 4 * _NS_MM512
        v_ns = 4 * 2 * 4 * _NS_MM256
        y_ns = 8 * 2 * _NS_MM512
        fills = [
            ((wave_units(1, parts=("v",)), wave_units(2, parts=("qt",))),
             v_ns + qk_ns // 2, ()),
            ((wave_units(2, parts=("kt",)), wave_units(2, parts=("v",))),
             qk_ns // 2 + v_ns, ()),
            ((wave_units(3, parts=("qt", "kt")), out_proj_units(0)),
             qk_ns + y_ns, ()),
            ((wave_units(3, parts=("v",)), out_proj_units(1),
              out_proj_units(2, sbs=(8, 9)), y3_dc0_units()),
             v_ns + y_ns + 4 * _NS_MM512 + 8 * _NS_MM512,
             ((2, 1000), (5, 1000), (8, 1000))),
        ]
        for qb in range(NQB):
            gens, n_fill, boosts = fills[qb]

            def _chain(gs=tuple(gens)):
                for g in gs:
                    yield from g
            attention_block(qb, _chain(), n_fill, boosts=boosts)
        for _ in out_proj_units(2, sbs=(10, 11)):
            pass
        for _ in y3_dc1_units():
            pass

    if loop_n:
        # bench-only path: hint all engines so the back-edge prefetches
        # the body's IRAM blocks (body >256 instructions per engine)
        hints = (mybir.EngineType.PE, mybir.EngineType.Activation,
                 mybir.EngineType.DVE, mybir.EngineType.SP,
                 mybir.EngineType.Pool)
        with tc.For_i(0, loop_n, 1, hint_engines=hints):
            _full_body()
    else:
        _full_body()


def _get_program(loop_n=0):
    key = ("nc", loop_n)
    if key in _CACHE:
        return _CACHE[key]
    import concourse.tile as tile
    from concourse import bacc, mybir

    f16 = mybir.dt.float16
    nc = bacc.Bacc("TRN2", target_bir_lowering=False, debug=False,
                   enable_asserts=False)
    xT = nc.dram_tensor("xT", [P, EC, S], f16, kind="ExternalInput").ap()
    wq = nc.dram_tensor("wq", [P, EC, DL], f16, kind="ExternalInput").ap()
    wk = nc.dram_tensor("wk", [P, EC, DL], f16, kind="ExternalInput").ap()
    wv = nc.dram_tensor("wv", [P, EC, DL], f16, kind="ExternalInput").ap()
    wo = nc.dram_tensor("wo", [P, MT, E], f16, kind="ExternalInput").ap()
    mask = nc.dram_tensor("mask", [P, P], f16, kind="ExternalInput").ap()
    y = nc.dram_tensor("y", [S, E], f16, kind="ExternalOutput").ap()
    y2 = nc.dram_tensor("y2", [512, E], f16, kind="ExternalOutput").ap()
    with tile.TileContext(nc) as tc:
        with ExitStack() as ctx:
            _emit(ctx, tc, xT, wq, wk, wv, wo, mask, y, y2, loop_n=loop_n)
    nc.compile()
    _CACHE[key] = nc
    return nc


def _pmajor(a):
    # [chunks*P, inner] -> [P, chunks, inner] partition-major layout
    a = np.ascontiguousarray(a)
    chunks = a.shape[0] // P
    return np.ascontiguousarray(
        a.reshape(chunks, P, *a.shape[1:]).transpose(1, 0, 2)).astype(
            np.float16)


def _make_in_maps(x, Wq, Wk, Wv, Wo):
    x = np.asarray(x, dtype=np.float32)
    Wq = np.asarray(Wq, dtype=np.float32)
    Wk = np.asarray(Wk, dtype=np.float32)
    Wv = np.asarray(Wv, dtype=np.float32)
    Wo = np.asarray(Wo, dtype=np.float32)
    mask = np.triu(np.ones((P, P), dtype=np.float16))
    in_maps = []
    for c in range(N_CORES):
        b, hg = divmod(c, CPB)
        hs = slice(hg * HL, (hg + 1) * HL)
        in_maps.append({
            "xT": _pmajor(x[b].T),
            "wq": _pmajor(Wq.reshape(E, H, D)[:, hs, :].reshape(E, DL)),
            "wk": _pmajor(Wk.reshape(E, H, D)[:, hs, :].reshape(E, DL)),
            "wv": _pmajor(Wv.reshape(E, H, D)[:, hs, :].reshape(E, DL)),
            "wo": _pmajor(Wo.reshape(H, D, E)[hs, :, :].reshape(DL, E)),
            "mask": mask,
        })
    return in_maps


def run(x, Wq, Wk, Wv, Wo, trace=False):
    from concourse.bass_utils import run_bass_kernel_spmd

    nc = _get_program()
    in_maps = _make_in_maps(x, Wq, Wk, Wv, Wo)
    br = run_bass_kernel_spmd(nc, in_maps, list(range(N_CORES)), trace=trace)
    out = np.zeros((B, S, E), dtype=np.float32)
    for c in range(N_CORES):
        out[c // CPB] += br.results[c]["y"].astype(np.float32)
        out[c // CPB][S - 512:] += br.results[c]["y2"].astype(np.float32)
    return out, br


def kernel(x, Wq, Wk, Wv, Wo):
    out, _ = run(x, Wq, Wk, Wv, Wo, trace=False)
    return out


# revision 36
# speedup vs baseline: 1.0012x; 1.0012x over previous
"""Causal self-attention (B=2, S=2048, E=1024, H=16) on 8 Trainium2 cores.

Sharding: core c in 0..7 handles batch b = c//4 and the 4 heads
[4*(c%4), 4*(c%4)+4).  The host pre-transposes x[b] and pre-slices the
QKV weights column-wise / Wo row-wise per core (already in the SBUF
partition-major layout the kernel wants); each core computes its heads'
attention plus its partial output projection, and the host sums the 4
f16 partials per batch.

Device kernel (per core, everything resident in SBUF, matmul inputs in
fp16 with fp32 PSUM accumulation):
  Inputs stream in over three DMA queues (x col-blocks alternate SP/ACT
  HWDGE, weights via Pool SWDGE, first e-chunk first) so the first
  projection chain starts ~3us in and PE never re-enters the slow
  p-state: wave 0 runs 8 concurrent QT/KT chains (q-blocks 0+1,
  borrowing the idle attention PSUM banks) ec-major against the
  arriving x blocks, then V[0..3].
  S^T tiles = matmul(lhsT=KT_blk, rhs=QT_blk): k on partitions, q free;
  exp on ScalarE (1/sqrt(D) folded into the activation scale); causal
  masking = never computing strictly-below-diagonal column ranges plus
  one 128x128 triangular mask multiply per diagonal block.
  P^T @ V with V augmented by a ones column (softmax denominator falls
  out of the same accumulation); normalization = copy O^T|l to SBUF
  (frees PSUM early), reciprocal on DVE, partition-broadcast of 1/l on
  the otherwise-idle Pool engine, one DVE multiply.
  Y = O @ Wo streamed out per q-block as f16 (host upcasts and sums).
  Fill (remaining waves / prev Y chains) is deficit-paced between the S
  and PV pairs of every kb-step, with an extra dose at head-pair
  boundaries to cover the O-PSUM handoff; the last q-block's Y is split
  so its first half runs as late-attention fill and two Y(2) s-blocks
  are held back to cover the final normalize chain.
"""

import numpy as np
from contextlib import ExitStack

B, S, E, H, D = 2, 2048, 1024, 16, 64
N_CORES = 8
CPB = 4              # cores per batch
HL = H // CPB        # heads per core = 4
DL = HL * D          # local head dims = 256
P = 128              # partitions
EC = E // P          # 8 e-chunks
SB = S // P          # 16 s/k blocks
NQB = S // 512       # 4 q blocks of 512
MT = DL // P         # 2 row-tiles of QT/KT/OT (2 heads each)

_CACHE = {}
_EXHAUSTED = object()

# rough PE-ns unit costs for fill pacing (matmul ~ N * 0.4167ns)
_NS_MM512 = 213
_NS_MM256 = 107


def _emit(ctx, tc, xT, wq, wk, wv, wo, mask, y, y2, loop_n=0):
    import concourse.bass as bass  # noqa: F401
    from concourse import mybir

    nc = tc.nc
    f32 = mybir.dt.float32
    f16 = mybir.dt.float16
    Exp = mybir.ActivationFunctionType.Exp

    res = ctx.enter_context(tc.tile_pool(name="res", bufs=1))
    xt_sb = res.tile([P, EC, S], f16, tag="xt")
    wq_sb = res.tile([P, EC, DL], f16, tag="wq")
    wk_sb = res.tile([P, EC, DL], f16, tag="wk")
    wv_sb = res.tile([P, EC, DL], f16, tag="wv")
    wo_sb = res.tile([P, MT, E], f16, tag="wo")
    qt_sb = res.tile([P, MT, S], f16, tag="qt")
    kt_sb = res.tile([P, MT, S], f16, tag="kt")
    vt_sb = res.tile([P, SB, HL, D + 1], f16, tag="vt")
    ot_sb = res.tile([P, MT, S], f16, tag="ot")
    ot3_sb = res.tile([P, MT, 512], f16, tag="ot3")
    mask_sb = res.tile([P, P], f16, tag="mask")

    mm_ps = ctx.enter_context(tc.tile_pool(name="mm", bufs=2, space="PSUM"))
    s_ps = ctx.enter_context(tc.tile_pool(name="sps", bufs=2, space="PSUM"))
    o_ps = ctx.enter_context(tc.tile_pool(name="ops", bufs=2, space="PSUM"))

    e_pool = ctx.enter_context(tc.tile_pool(name="ep", bufs=6))
    y_pool = ctx.enter_context(tc.tile_pool(name="yp", bufs=8))
    n_pool = ctx.enter_context(tc.tile_pool(name="np", bufs=8))

    def _full_body():
        sp, act, gp = nc.sync, nc.scalar, nc.gpsimd
        tail_norm = {}

        # ---- input DMAs: weights on Pool SWDGE (first e-chunks first so
        # the projection chains can start), x col-blocks on SP/ACT HWDGE,
        # ec-major for q-blocks 0/1 feeding wave 0, then q-blocks 2/3. ----
        act.dma_start(out=wk_sb[:, 0:1, :], in_=wk[:, 0:1, :])
        act.dma_start(out=wk_sb[:, 1:4, :], in_=wk[:, 1:4, :])
        gp.dma_start(out=wq_sb[:, 0:1, :], in_=wq[:, 0:1, :])
        gp.dma_start(out=wq_sb[:, 1:4, :], in_=wq[:, 1:4, :])
        gp.dma_start(out=wq_sb[:, 4:8, :], in_=wq[:, 4:8, :])
        gp.dma_start(out=wk_sb[:, 4:8, :], in_=wk[:, 4:8, :])
        gp.dma_start(out=wv_sb[:], in_=wv[:])
        gp.dma_start(out=mask_sb[:], in_=mask[:])
        gp.dma_start(out=wo_sb[:], in_=wo[:])
        for ec in range(EC):
            sp.dma_start(out=xt_sb[:, ec, 0:512], in_=xT[:, ec, 0:512])
            act.dma_start(out=xt_sb[:, ec, 512:1024], in_=xT[:, ec, 512:1024])
        for nb in (2, 3):
            for ec in range(EC):
                eng = sp if ec % 2 == 0 else act
                eng.dma_start(out=xt_sb[:, ec, nb * 512:(nb + 1) * 512],
                              in_=xT[:, ec, nb * 512:(nb + 1) * 512])
        nc.vector.memset(vt_sb[:, :, :, D:D + 1], 1.0)

        def wave_units(nb, parts=("qt", "kt", "v")):
            # QT/KT [:, :, nb-window] = (w chunk)^T @ xT ; V[4nb..4nb+3].
            # Generator yielding PE-ns cost estimates between chunks so the
            # attention weave can deficit-pace its fill.
            srcs = []
            if "qt" in parts:
                srcs.append((wq_sb, qt_sb))
            if "kt" in parts:
                srcs.append((wk_sb, kt_sb))
            for mt in range(MT):
                for w_sb, t_sb in srcs:
                    ps = mm_ps.tile([P, 512], f32, tag="mm", name="wv_ps")
                    for ec in range(EC):
                        nc.tensor.matmul(
                            ps[:],
                            w_sb[:, ec, mt * P:(mt + 1) * P],
                            xt_sb[:, ec, nb * 512:(nb + 1) * 512],
                            start=(ec == 0), stop=(ec == EC - 1))
                        if ec == 3:
                            yield 4 * _NS_MM512
                    nc.vector.tensor_copy(
                        t_sb[:, mt, nb * 512:(nb + 1) * 512], ps[:])
                    yield 4 * _NS_MM512
            if "v" not in parts:
                return
            for sb in range(4 * nb, 4 * nb + 4):
                ps = mm_ps.tile([P, 512], f32, tag="mm", name="v_ps")
                for ec in range(EC):
                    nc.tensor.matmul(
                        ps[:, 0:DL],
                        xt_sb[:, ec, sb * P:(sb + 1) * P],
                        wv_sb[:, ec, :],
                        start=(ec == 0), stop=(ec == EC - 1))
                    if ec == 3:
                        yield 4 * _NS_MM256
                nc.vector.tensor_copy(
                    vt_sb[:, sb, :, 0:D],
                    ps[:, 0:DL].rearrange("p (h d) -> p h d", h=HL))
                yield 4 * _NS_MM256

        def out_proj_units(qb, sbs=None):
            # Y[sb, :] = O[sb, :] @ wo, staged to f16, one DMA per s-block.
            if sbs is None:
                sbs = range(4 * qb, 4 * qb + 4)
            for sb in sbs:
                yt = y_pool.tile([P, E], f16, tag="y", name="yt")
                for eb in range(E // 512):
                    yp = mm_ps.tile([P, 512], f32, tag="mm", name="yp")
                    for dc in range(MT):
                        nc.tensor.matmul(
                            yp[:],
                            ot_sb[:, dc, sb * P:(sb + 1) * P],
                            wo_sb[:, dc, eb * 512:(eb + 1) * 512],
                            start=(dc == 0), stop=(dc == MT - 1))
                    nc.vector.tensor_copy(
                        yt[:, eb * 512:(eb + 1) * 512], yp[:])
                    yield 2 * _NS_MM512
                sp.dma_start(out=y[sb * P:(sb + 1) * P, :], in_=yt[:])

        def y3_dc0_units():
            # head-pair-0 half of the last q-block's output projection
            # (ready after attn(3)'s first mt pass) ships to the y2 partial
            # output, which the host sums into y; only the head-pair-1
            # matmuls remain for the tail.
            for sb in range(12, 16):
                for eb in range(2):
                    yt = y_pool.tile([P, 512], f16, tag="y", name="y2t")
                    yp = mm_ps.tile([P, 512], f32, tag="mm", name="yp0")
                    nc.tensor.matmul(
                        yp[:],
                        ot3_sb[:, 0, (sb - 12) * P:(sb - 11) * P],
                        wo_sb[:, 0, eb * 512:(eb + 1) * 512],
                        start=True, stop=True)
                    nc.vector.tensor_copy(yt[:], yp[:])
                    sp.dma_start(
                        out=y2[(sb - 12) * P:(sb - 11) * P,
                               eb * 512:(eb + 1) * 512],
                        in_=yt[:])
                    yield _NS_MM512

        def y3_dc1_units():
            Copy = mybir.ActivationFunctionType.Copy
            ops3, rbcs = tail_norm["ops"], tail_norm["rbcs"]
            for sb in range(12, 16):
                cs = slice((sb - 12) * P, (sb - 11) * P)
                for half in range(2):
                    dr = half * D
                    nc.vector.tensor_mul(
                        ot3_sb[dr:dr + D, 1, cs],
                        ops3[half][0:D, cs], rbcs[half][0:D, cs])
                yt = y_pool.tile([P, E], f16, tag="y", name="yt3")
                for eb in range(2):
                    yp = mm_ps.tile([P, 512], f32, tag="mm", name="yp1")
                    nc.tensor.matmul(
                        yp[:],
                        ot3_sb[:, 1, (sb - 12) * P:(sb - 11) * P],
                        wo_sb[:, 1, eb * 512:(eb + 1) * 512],
                        start=True, stop=True)
                    dst = yt[:, eb * 512:(eb + 1) * 512]
                    if eb == 0:
                        nc.scalar.activation(out=dst, in_=yp[:], func=Copy)
                    else:
                        nc.vector.tensor_copy(dst, yp[:])
                    sp.dma_start(
                        out=y[sb * P:(sb + 1) * P, eb * 512:(eb + 1) * 512],
                        in_=dst)
                    yield _NS_MM512

        def attention_block(qb, fill_units, fill_ns, boosts=()):
            # ACT-paced kb-steps; fill (PE work with no dependence on this
            # block) is inserted between the S pair and the PV pair of every
            # step, deficit-paced so the reservoir lasts the whole block;
            # head-pair boundaries get an extra dose to cover the O-PSUM
            # handoff, and `boosts` front-loads deadline-constrained units
            # (e.g. this block's own late V tiles).
            nkb = 4 * (qb + 1)     # causal: k blocks 0 .. nkb-1
            scale = float(1.0 / np.sqrt(D))
            nsteps = MT * nkb
            boosts = dict(boosts)
            committed = MT * 1500 + MT * 2 * 1100 + sum(boosts.values())
            per_step = max(0.0, fill_ns - committed) / max(1, nsteps - 4)
            state = {"due": 0.0, "done": False}

            def run_fill(budget):
                state["due"] += budget
                while state["due"] > 0 and not state["done"]:
                    got = next(fill_units, _EXHAUSTED)
                    if got is _EXHAUSTED:
                        state["done"] = True
                        break
                    state["due"] -= got

            for mt in range(MT):   # head pair (2*mt, 2*mt+1)
                op0 = o_ps.tile([P, 512], f32, tag="o", name="op0")
                op1 = o_ps.tile([P, 512], f32, tag="o", name="op1")
                ops = [op0, op1]
                for kb in range(nkb):
                    t = kb - 4 * qb
                    v0 = P * t if t > 0 else 0   # masked prefix of window
                    sp_t = s_ps.tile([P, 1024], f32, tag="s", name="sp_t")
                    for half in range(2):
                        dr = half * D
                        nc.tensor.matmul(
                            sp_t[:, half * 512 + v0:(half + 1) * 512],
                            kt_sb[dr:dr + D, mt, kb * P:(kb + 1) * P],
                            qt_sb[dr:dr + D, mt,
                                  qb * 512 + v0:(qb + 1) * 512],
                            start=True, stop=True)
                    et = e_pool.tile([P, 1024], f16, tag="e", name="et")
                    nc.scalar.activation(out=et[:, v0:], in_=sp_t[:, v0:],
                                         func=Exp, scale=scale)
                    if t >= 0:  # diagonal block: mask strictly-future keys
                        for half in range(2):
                            w0 = half * 512 + v0
                            nc.vector.tensor_mul(
                                et[:, w0:w0 + P], et[:, w0:w0 + P],
                                mask_sb[:])
                    step = mt * nkb + kb
                    run_fill(per_step + (1500 if kb == 0 else 0)
                             + boosts.get(step, 0))
                    for half in range(2):
                        nc.tensor.matmul(
                            ops[half][0:D + 1, v0:],
                            vt_sb[:, kb, 2 * mt + half, :],
                            et[:, half * 512 + v0:(half + 1) * 512],
                            start=(kb == 0), stop=(kb == nkb - 1))
                # normalize: copy O^T|l off PSUM (frees the o tiles fast),
                # 1/l on DVE, partition-broadcast on Pool, one DVE multiply.
                last = (qb == NQB - 1)
                if last and mt == MT - 1:
                    # tail-critical: 1/l straight from the O PSUM (held to
                    # the end -- nothing else needs the banks), partition-
                    # broadcast to SBUF on Pool; the per-s-block muls are
                    # emitted by the dc1 units so each unblocks as soon as
                    # its own columns are normalized.
                    rbcs = []
                    for half in range(2):
                        rec = n_pool.tile([1, 512], f32, tag="rec",
                                          name="rec")
                        nc.vector.reciprocal(rec[:], ops[half][D:D + 1, :])
                        rbc = n_pool.tile([D, 512], f32, tag="rbc",
                                          name="rbc")
                        nc.gpsimd.partition_broadcast(rbc[:], rec[:])
                        rbcs.append(rbc)
                    tail_norm["ops"] = ops
                    tail_norm["rbcs"] = rbcs
                    continue
                for half in range(2):
                    op = ops[half]
                    dr = half * D
                    otr = n_pool.tile([D + 1, 512], f32, tag="otr",
                                      name="otr")
                    if last:
                        nc.scalar.activation(
                            out=otr[:], in_=op[0:D + 1, :],
                            func=mybir.ActivationFunctionType.Copy)
                    else:
                        nc.vector.tensor_copy(otr[:], op[0:D + 1, :])
                    rec = n_pool.tile([1, 512], f32, tag="rec", name="rec")
                    nc.vector.reciprocal(rec[:], otr[D:D + 1, :])
                    rbc = n_pool.tile([D, 512], f32, tag="rbc", name="rbc")
                    nc.gpsimd.partition_broadcast(rbc[:], rec[:])
                    if last:
                        dst = ot3_sb[dr:dr + D, mt, :]
                    else:
                        dst = ot_sb[dr:dr + D, mt, qb * 512:(qb + 1) * 512]
                    nc.vector.tensor_mul(dst, otr[0:D, :], rbc[:])
                    run_fill(1100)
            # drain leftover fill
            while next(fill_units, _EXHAUSTED) is not _EXHAUSTED:
                pass

        # ---- wave 0, ec-major: 8 concurrent QT/KT chains (q-blocks 0+1;
        # accumulators borrowed from every PSUM pool) advance one e-chunk
        # at a time as x col-blocks land, keeping PE in deep backlog so the
        # p-state ramps once, then V[0..3].  q-block-0 copies retire on DVE
        # as their chains stop; the rest split across DVE and the
        # still-idle ACT engine so vt copies are never stuck behind them.
        Copy = mybir.ActivationFunctionType.Copy
        w0ps = []
        for mt in range(MT):
            w0ps.append((0, mt, wq_sb, qt_sb,
                         mm_ps.tile([P, 512], f32, tag="mm", name="w0qa")))
        for mt in range(MT):
            w0ps.append((0, mt, wk_sb, kt_sb,
                         o_ps.tile([P, 512], f32, tag="o", name="w0ka")))
        sA = s_ps.tile([P, 1024], f32, tag="s", name="w0sA")
        sB = s_ps.tile([P, 1024], f32, tag="s", name="w0sB")
        for mt in range(MT):
            w0ps.append((1, mt, wq_sb, qt_sb, sA[:, mt * 512:(mt + 1) * 512]))
        for mt in range(MT):
            w0ps.append((1, mt, wk_sb, kt_sb, sB[:, mt * 512:(mt + 1) * 512]))
        for ec in range(EC):
            for i, (nb, mt, w_sb, t_sb, pchain) in enumerate(w0ps):
                nc.tensor.matmul(
                    pchain[:],
                    w_sb[:, ec, mt * P:(mt + 1) * P],
                    xt_sb[:, ec, nb * 512:(nb + 1) * 512],
                    start=(ec == 0), stop=(ec == EC - 1))
                if ec == EC - 1:
                    nb_, mt_, _, t_sb_, pch_ = w0ps[i]
                    dst = t_sb_[:, mt_, nb_ * 512:(nb_ + 1) * 512]
                    if i in (0, 1, 4, 5):
                        nc.vector.tensor_copy(dst, pch_[:])
                    else:
                        nc.scalar.activation(out=dst, in_=pch_[:], func=Copy)
        for sb in range(4):
            ps = mm_ps.tile([P, 512], f32, tag="mm", name="w0v")
            for ec in range(EC):
                nc.tensor.matmul(
                    ps[:, 0:DL],
                    xt_sb[:, ec, sb * P:(sb + 1) * P],
                    wv_sb[:, ec, :],
                    start=(ec == 0), stop=(ec == EC - 1))
            nc.vector.tensor_copy(
                vt_sb[:, sb, :, 0:D],
                ps[:, 0:DL].rearrange("p (h d) -> p h d", h=HL))

        # Fill plan, sized to each block's ACT-over-PE deficit (which grows
        # with qb): attention(3) gets its own late V tiles (front-loaded via
        # boosts to beat the kb=12 deadline), Y(1), the first half of Y(2),
        # and Y(3)'s dc0 partials; Y(2)'s last two s-blocks plus the dc1
        # finish run in the tail so the final normalize chain has PE work
        # under it.
        qk_ns = 4 * 4 * _NS_MM512
        v_ns = 4 * 2 * 4 * _NS_MM256
        y_ns = 8 * 2 * _NS_MM512
        fills = [
            ((wave_units(1, parts=("v",)), wave_units(2, parts=("qt",))),
             v_ns + qk_ns // 2, ()),
            ((wave_units(2, parts=("kt",)), wave_units(2, parts=("v",))),
             qk_ns // 2 + v_ns, ()),
            ((wave_units(3, parts=("qt", "kt")), out_proj_units(0)),
             qk_ns + y_ns, ()),
            ((wave_units(3, parts=("v",)), out_proj_units(1),
              out_proj_units(2, sbs=(8, 9)), y3_dc0_units()),
             v_ns + y_ns + 4 * _NS_MM512 + 8 * _NS_MM512,
             ((2, 300), (5, 300), (8, 300))),
        ]
        for qb in range(NQB):
            gens, n_fill, boosts = fills[qb]

            def _chain(gs=tuple(gens)):
                for g in gs:
                    yield from g
            attention_block(qb, _chain(), n_fill, boosts=boosts)
        for _ in out_proj_units(2, sbs=(10, 11)):
            pass
        for _ in y3_dc1_units():
            pass

    if loop_n:
        # bench-only path: hint all engines so the back-edge prefetches
        # the body's IRAM blocks (body >256 instructions per engine)
        hints = (mybir.EngineType.PE, mybir.EngineType.Activation,
                 mybir.EngineType.DVE, mybir.EngineType.SP,
                 mybir.EngineType.Pool)
        with tc.For_i(0, loop_n, 1, hint_engines=hints):
            _full_body()
    else:
        _full_body()


def _get_program(loop_n=0):
    key = ("nc", loop_n)
    if key in _CACHE:
        return _CACHE[key]
    import concourse.tile as tile
    from concourse import bacc, mybir

    f16 = mybir.dt.float16
    nc = bacc.Bacc("TRN2", target_bir_lowering=False, debug=False,
                   enable_asserts=False)
    xT = nc.dram_tensor("xT", [P, EC, S], f16, kind="ExternalInput").ap()
    wq = nc.dram_tensor("wq", [P, EC, DL], f16, kind="ExternalInput").ap()
    wk = nc.dram_tensor("wk", [P, EC, DL], f16, kind="ExternalInput").ap()
    wv = nc.dram_tensor("wv", [P, EC, DL], f16, kind="ExternalInput").ap()
    wo = nc.dram_tensor("wo", [P, MT, E], f16, kind="ExternalInput").ap()
    mask = nc.dram_tensor("mask", [P, P], f16, kind="ExternalInput").ap()
    y = nc.dram_tensor("y", [S, E], f16, kind="ExternalOutput").ap()
    y2 = nc.dram_tensor("y2", [512, E], f16, kind="ExternalOutput").ap()
    with tile.TileContext(nc) as tc:
        with ExitStack() as ctx:
            _emit(ctx, tc, xT, wq, wk, wv, wo, mask, y, y2, loop_n=loop_n)
    nc.compile()
    _CACHE[key] = nc
    return nc


def _pmajor(a):
    # [chunks*P, inner] -> [P, chunks, inner] partition-major layout
    a = np.ascontiguousarray(a)
    chunks = a.shape[0] // P
    return np.ascontiguousarray(
        a.reshape(chunks, P, *a.shape[1:]).transpose(1, 0, 2)).astype(
            np.float16)


def _make_in_maps(x, Wq, Wk, Wv, Wo):
    x = np.asarray(x, dtype=np.float32)
    Wq = np.asarray(Wq, dtype=np.float32)
    Wk = np.asarray(Wk, dtype=np.float32)
    Wv = np.asarray(Wv, dtype=np.float32)
    Wo = np.asarray(Wo, dtype=np.float32)
    mask = np.triu(np.ones((P, P), dtype=np.float16))
    in_maps = []
    for c in range(N_CORES):
        b, hg = divmod(c, CPB)
        hs = slice(hg * HL, (hg + 1) * HL)
        in_maps.append({
            "xT": _pmajor(x[b].T),
            "wq": _pmajor(Wq.reshape(E, H, D)[:, hs, :].reshape(E, DL)),
            "wk": _pmajor(Wk.reshape(E, H, D)[:, hs, :].reshape(E, DL)),
            "wv": _pmajor(Wv.reshape(E, H, D)[:, hs, :].reshape(E, DL)),
            "wo": _pmajor(Wo.reshape(H, D, E)[hs, :, :].reshape(DL, E)),
            "mask": mask,
        })
    return in_maps


def run(x, Wq, Wk, Wv, Wo, trace=False):
    from concourse.bass_utils import run_bass_kernel_spmd

    nc = _get_program()
    in_maps = _make_in_maps(x, Wq, Wk, Wv, Wo)
    br = run_bass_kernel_spmd(nc, in_maps, list(range(N_CORES)), trace=trace)
    out = np.zeros((B, S, E), dtype=np.float32)
    for c in range(N_CORES):
        out[c // CPB] += br.results[c]["y"].astype(np.float32)
        out[c // CPB][S - 512:] += br.results[c]["y2"].astype(np.float32)
    return out, br


def kernel(x, Wq, Wk, Wv, Wo):
    out, _ = run(x, Wq, Wk, Wv, Wo, trace=False)
    return out
